# revision 28
# baseline (speedup 1.0000x reference)
"""Jacobi 100-step solver on 8 trn2 cores via truncated DST-spectral transform.

x_{t+1} = mask * (0.25 * 4-neighbor-sum) is linear; after one explicit step
(x1 has zero boundary) the dynamics diagonalize in the DST basis Q:
x100 = Q (s^99 (.) (Q^T x1 Q)) Q^T with s = 0.5(cos a + cos b). |s|^99 is
negligible outside the lowest-K and highest-K mode corners (K=256 -> rel err
~1e-2 incl. bf16, tol 2e-2), so only two [256,256] spectral blocks survive.

Sharding: forward is column-panel (256 cols + 1-col halos per core); one
bf16 AllReduce of the [512,256] stacked G^T blocks; backward is row-block
(each core produces 256 output rows). All matmuls bf16; Qc^T is
host-precomputed (no PE transposes); every DRAM input is host-pre-shaped to
SBUF layout [128, F] so each load is one wide contiguous DMA; a tiny dummy
AllGather at kernel start absorbs NRT's one-time collective bootstrap so the
real AllReduce runs warm.
"""

import sys
import types
import numpy as np

N = 2048
NC = 8
P = N // NC          # 256 panel cols (fwd) / block rows (bwd) per core
K = 256              # spectral corner size
PW = P + 2           # 258: panel width with 1-col halos
RC = N // 128        # 16 row chunks
SC = 4               # phase0 super-chunks (4 row chunks each)
TS = ("lo", "hi")


def _install_ntff_hook():
    if "antenv.axon_hooks" in sys.modules:
        return
    mod = types.ModuleType("antenv.axon_hooks")
    mod._hook = None
    mod.set_axon_ntff_profile_hook = lambda h: setattr(mod, "_hook", h)
    mod.get_axon_ntff_profile_hook = lambda: mod._hook
    sys.modules["antenv.axon_hooks"] = mod
    try:
        import antenv
        antenv.axon_hooks = mod
        from trn_agent_boot.trn_boot import _ntff_profile_via_ctypes
        h = _ntff_profile_via_ctypes("/opt/axon/libaxon_pjrt.so")
        if h is not None:
            mod.set_axon_ntff_profile_hook(h)
    except Exception:
        pass


def _to_sb(a, chunks):
    """[chunks*128, F] row-major -> SBUF layout [128, chunks*F]."""
    f = a.shape[1]
    return np.ascontiguousarray(
        a.reshape(chunks, 128, f).transpose(1, 0, 2).reshape(128, chunks * f))


def _host_constants():
    import ml_dtypes
    bf = ml_dtypes.bfloat16
    i = np.arange(N, dtype=np.float64)
    consts = {}
    for t in TS:
        m = (np.arange(1, K + 1, dtype=np.float64) if t == "lo"
             else np.arange(N - 1 - K, N - 1, dtype=np.float64))
        red = np.outer(i, m) % (2 * (N - 1))
        Qc = np.sqrt(2.0 / (N - 1)) * np.sin(np.pi * red / (N - 1))  # [2048, K]
        lam = 0.5 * np.cos(np.pi * m / (N - 1))
        W99 = (lam[:, None] + lam[None, :]) ** 99                    # [K, K] sym
        consts[f"qcf_{t}"] = Qc
        consts[f"qcT_{t}"] = _to_sb(np.ascontiguousarray(Qc.T).astype(bf), 2)
        consts[f"w99_{t}"] = _to_sb(W99.astype(bf), 2)
    # lo|hi interleaved per row-strip so mm1's moving operand is 512 wide
    consts["qcall"] = _to_sb(
        np.concatenate([consts["qcf_lo"], consts["qcf_hi"]], axis=1).astype(bf), RC)
    smid = np.zeros((128, 128), np.float64)
    for d in range(127):
        smid[d, d + 1] = 1.0
        smid[d + 1, d] = 1.0
    sup = np.zeros((128, 128), np.float64); sup[127, 0] = 1.0
    sdn = np.zeros((128, 128), np.float64); sdn[0, 127] = 1.0
    consts["smid"] = smid.astype(bf)
    consts["sup"] = sup.astype(bf)
    consts["sdn"] = sdn.astype(bf)
    return consts


_NC_CACHE = {}


def _build():
    if "nc" in _NC_CACHE:
        return _NC_CACHE["nc"]
    import concourse.bacc as bacc
    import concourse.tile as tile
    import concourse.mybir as mybir

    BF = mybir.dt.bfloat16
    F32 = mybir.dt.float32
    ACTF = mybir.ActivationFunctionType
    LN025 = float(np.log(0.25))

    nc = bacc.Bacc("TRN2", target_bir_lowering=False, debug=False, num_devices=NC)

    # all inputs host-pre-shaped to SBUF layout [128, F]
    xin = nc.dram_tensor("X", [128, RC * PW], BF, kind="ExternalInput")
    yin = nc.dram_tensor("Y", [128, RC * PW], BF, kind="ExternalInput")
    qcall_d = nc.dram_tensor("qcall", [128, RC * 2 * K], BF, kind="ExternalInput")
    qrows_d = {t: nc.dram_tensor(f"qrows_{t}", [128, 2 * K], BF, kind="ExternalInput") for t in TS}
    qrT_d = {t: nc.dram_tensor(f"qrT_{t}", [128, 2 * P], BF, kind="ExternalInput") for t in TS}
    qcT_d = {t: nc.dram_tensor(f"qcT_{t}", [128, 2 * N], BF, kind="ExternalInput") for t in TS}
    w99_d = {t: nc.dram_tensor(f"w99_{t}", [128, 2 * K], BF, kind="ExternalInput") for t in TS}
    smid_d = nc.dram_tensor("smid", [128, 128], BF, kind="ExternalInput")
    sup_d = nc.dram_tensor("sup", [128, 128], BF, kind="ExternalInput")
    sdn_d = nc.dram_tensor("sdn", [128, 128], BF, kind="ExternalInput")
    out_d = nc.dram_tensor("out", [128, 2 * N], F32, kind="ExternalOutput")

    with tile.TileContext(nc) as tc:
        with tc.tile_pool(name="pers", bufs=1) as pers, \
             tc.tile_pool(name="rot", bufs=2) as rot, \
             tc.tile_pool(name="dram", bufs=1, space="DRAM") as dram:

            # ---- persistent SBUF ----
            xall = pers.tile([128, RC * PW], BF, tag="xall")
            yall = pers.tile([128, RC * PW], BF, tag="yall")
            x0b = pers.tile([128, RC * PW], BF, tag="x0b")
            x1b = pers.tile([128, RC * P], BF, tag="x1b")
            qcall_s = pers.tile([128, RC * 2 * K], BF, tag="qcall")
            qrows_s = {t: pers.tile([128, 2 * K], BF, tag=f"qr{t}", name=f"qr_{t}") for t in TS}
            qrT_s = {t: pers.tile([128, 2 * P], BF, tag=f"qx{t}", name=f"qx_{t}") for t in TS}
            qcT_s = {t: pers.tile([128, 2 * N], BF, tag=f"qt{t}", name=f"qt_{t}") for t in TS}
            w99_s = {t: pers.tile([128, 2 * K], BF, tag=f"w9{t}", name=f"w9_{t}") for t in TS}
            a_all = {jm: pers.tile([128, 2 * K], BF, tag=f"aa{jm}", name=f"aa_{jm}") for jm in range(2)}
            ut_s = {t: pers.tile([128, 2 * K], BF, tag=f"ut{t}", name=f"ut_{t}") for t in TS}
            cp_s = {t: pers.tile([128, 2 * P], BF, tag=f"cp{t}", name=f"cp_{t}") for t in TS}
            outs = pers.tile([128, 2 * N], F32, tag="outs")
            smid_s = pers.tile([128, 128], BF, tag="smid")
            sup_s = pers.tile([128, 128], BF, tag="sup")
            sdn_s = pers.tile([128, 128], BF, tag="sdn")

            # ---- const APs for activation bias values + early ACT table load ----
            for cv, cn in ((-0.5, "cneg05"), (LN025, "cln025")):
                ct = pers.tile([128, 1], F32, tag=cn, name=cn)
                nc.vector.memset(ct[:], cv)
                nc.const_aps.aps[(F32, float(cv))] = ct[:]
            wact = pers.tile([128, 1], F32, tag="wact")
            nc.scalar.activation(wact[:], nc.const_aps.aps[(F32, -0.5)], ACTF.Square,
                                 bias=-0.5, scale=1.0)
            nc.scalar.activation(wact[:], wact[:], ACTF.Exp, bias=LN025, scale=-50.0)

            # ---- input DMAs (all contiguous [128, F] copies); X/Y then qcall
            # on sync, small consts on gpsimd — no queue saturates ----
            nc.sync.dma_start(smid_s[:], smid_d[:, :])
            nc.sync.dma_start(sup_s[:], sup_d[:, :])
            nc.sync.dma_start(sdn_s[:], sdn_d[:, :])
            W0 = 4 * PW
            for s in range(SC):
                nc.sync.dma_start(xall[:, W0 * s:W0 * (s + 1)], xin[:, W0 * s:W0 * (s + 1)])
                nc.sync.dma_start(yall[:, W0 * s:W0 * (s + 1)], yin[:, W0 * s:W0 * (s + 1)])
            for h in range(2):
                HK = RC * K  # half of qcall
                nc.sync.dma_start(qcall_s[:, HK * h:HK * (h + 1)],
                                  qcall_d[:, HK * h:HK * (h + 1)])
            for t in TS:
                nc.gpsimd.dma_start(qrows_s[t][:], qrows_d[t][:, :])
                nc.gpsimd.dma_start(w99_s[t][:], w99_d[t][:, :])
                nc.gpsimd.dma_start(qrT_s[t][:], qrT_d[t][:, :])

            # ---- phase 0: x0 = 0.25*exp(-50((X-.5)^2+(Y-.5)^2)) ----
            # scalar: Square(x)+Exp; vector: (y-.5)^2; gpsimd: the sum.
            W2 = 4 * PW
            for s in range(SC):
                xs = slice(W2 * s, W2 * (s + 1))
                sqt = rot.tile([128, W2], BF, tag="sqt", name="sqt")
                dt = rot.tile([128, W2], BF, tag="dt", name="dt")
                st = rot.tile([128, W2], BF, tag="st", name="st")
                nc.scalar.activation(sqt[:], xall[:, xs], ACTF.Square, bias=-0.5, scale=1.0)
                nc.vector.tensor_scalar_add(dt[:], yall[:, xs], -0.5)
                nc.vector.tensor_mul(dt[:], dt[:], dt[:])
                nc.gpsimd.tensor_add(st[:], sqt[:], dt[:])
                nc.scalar.activation(x0b[:, xs], st[:], ACTF.Exp, bias=LN025, scale=-50.0)

            from concourse.ap import AP as _AP

            def pair_mov(r, n=2):
                """[128, n, 256] moving AP over chunks r..r+n-1 center cols."""
                base = x0b[:, PW * r + 1:PW * r + 1 + P]
                rows = [list(rr) for rr in base.ap]
                return _AP(base.tensor, base.offset, [rows[0], [PW, n], [1, P]])

            with tc.tile_pool(name="psf", space="PSUM", bufs=1) as psf:
                # ---- phase 1 (one explicit Jacobi step) + mm1, per chunk pair.
                # Stencil matmuls are 512 wide (2 chunks per instr via 3D AP);
                # mm1 is 512 wide (lo|hi interleaved qcall). Emission staggers
                # mm1(pair-1) after stencil(pair) so PE never waits on the
                # vector x1 add of the current pair. ----
                aps = {jm: psf.tile([128, 2 * K], F32, tag=f"aps{jm}", bufs=1,
                                    name=f"aps_{jm}")
                       for jm in range(2)}

                def stencil(pr):
                    r0 = 2 * pr
                    vps = psf.tile([128, 2 * P], F32, tag="vps", bufs=2, name="vps")
                    mms = []
                    mms.append((smid_s, pair_mov(r0), None))
                    if pr > 0:
                        mms.append((sup_s, pair_mov(r0 - 1), None))
                    else:
                        mms.append((sup_s, x0b[:, PW * r0 + 1:PW * r0 + 1 + P], (P, 2 * P)))
                    if pr < RC // 2 - 1:
                        mms.append((sdn_s, pair_mov(r0 + 1), None))
                    else:
                        mms.append((sdn_s, x0b[:, PW * (r0 + 1) + 1:PW * (r0 + 1) + 1 + P], (0, P)))
                    for mi, (mat, mov, half) in enumerate(mms):
                        dst = vps[:] if half is None else vps[:, half[0]:half[1]]
                        nc.tensor.matmul(dst, mat[:], mov,
                                         start=(mi == 0), stop=(mi == len(mms) - 1),
                                         skip_group_check=True)
                    th = rot.tile([128, 2 * P], F32, tag="th", name="th")
                    thL = x0b[:, PW * r0:PW * r0 + P]
                    rows = [list(rr) for rr in thL.ap]
                    nc.gpsimd.tensor_add(
                        th[:],
                        _AP(thL.tensor, thL.offset, [rows[0], [PW, 2], [1, P]]),
                        _AP(thL.tensor, thL.offset + 2, [rows[0], [PW, 2], [1, P]]))
                    nc.vector.tensor_add(x1b[:, P * r0:P * (r0 + 2)], th[:], vps[:])

                def mm1(pr):
                    for q in range(2):
                        r = 2 * pr + q
                        for jm in range(2):
                            stat = x1b[:, P * r + 128 * jm:P * r + 128 * (jm + 1)]
                            nc.tensor.matmul(aps[jm][:], stat,
                                             qcall_s[:, 2 * K * r:2 * K * (r + 1)],
                                             start=(r == 0), stop=(r == RC - 1))

                for pr in range(RC // 2):
                    stencil(pr)
                    if pr > 0:
                        mm1(pr - 1)
                mm1(RC // 2 - 1)

                # ---- A evac (psum -> sbuf bf16), split scalar/vector ----
                nc.vector.tensor_copy(a_all[0][:], aps[0][:])
                nc.scalar.copy(a_all[1][:], aps[1][:])

                # ---- mm2: G_t[ka] += A_t[jm][ka-chunk]^T @ qrows[jm]; -> gin ----
                # (a-major orientation so the backward needs no transposes;
                # gin/gout kept in SBUF layout [128, 1024] so the AR payload
                # round-trips through DRAM as single contiguous DMAs)
                gin = dram.tile([128, 4 * K], BF, tag="gin")
                gout = dram.tile([128, 4 * K], BF, tag="gout", addr_space="Shared")
                for ti, t in enumerate(TS):
                    gps = psf.tile([128, 2 * K], F32, tag="vps", bufs=2, name="gps")
                    for ka in range(2):
                        for jm in range(2):
                            nc.tensor.matmul(gps[:, K * ka:K * (ka + 1)],
                                             a_all[jm][:, K * ti + 128 * ka:K * ti + 128 * (ka + 1)],
                                             qrows_s[t][:, K * jm:K * (jm + 1)],
                                             start=(jm == 0), stop=(jm == 1))
                    gsb = rot.tile([128, 2 * K], BF, tag="gsb", name="gsb")
                    if ti == 0:
                        nc.vector.tensor_copy(gsb[:], gps[:])
                    else:
                        nc.scalar.copy(gsb[:], gps[:])
                    nc.sync.dma_start(gin[:, 2 * K * ti:2 * K * (ti + 1)], gsb[:])

            # ---- AllReduce (bf16, 256KB) ----
            nc.gpsimd.collective_compute(
                "AllReduce", mybir.AluOpType.add,
                replica_groups=[list(range(NC))],
                ins=[gin.opt()], outs=[gout.opt()],
            )

            # ---- qcT strips load (lands during AR) ----
            for t in TS:
                nc.sync.dma_start(qcT_s[t][:], qcT_d[t][:, :])

            # ---- filter: Uhat = w99 (.) G (one contiguous gout DMA) ----
            gob = rot.tile([128, 4 * K], BF, tag="gob", name="gob")
            nc.sync.dma_start(gob[:], gout[:, :])
            nc.vector.tensor_mul(ut_s["lo"][:], gob[:, 0:2 * K], w99_s["lo"][:])
            nc.vector.tensor_mul(ut_s["hi"][:], gob[:, 2 * K:4 * K], w99_s["hi"][:])

            with tc.tile_pool(name="psb", space="PSUM", bufs=1) as psb:
                # ---- C: C_t^T[kb] = sum_ka Uhat_t[ka][:,kb]^T @ qrT_t[ka] ----
                # small side first: [256,256]@[256,256] per t — this replaces
                # the old fully-redundant Yhat = Uhat @ QcT (8x the FLOPs)
                for ti, t in enumerate(TS):
                    cps = psb.tile([128, 2 * P], F32, tag=f"cps{t}", bufs=1,
                                   name=f"cps_{t}")
                    for kb in range(2):
                        for ka in range(2):
                            nc.tensor.matmul(cps[:, P * kb:P * (kb + 1)],
                                             ut_s[t][:, K * ka + 128 * kb:K * ka + 128 * (kb + 1)],
                                             qrT_s[t][:, P * ka:P * (ka + 1)],
                                             start=(ka == 0), stop=(ka == 1))
                    if ti == 0:
                        nc.vector.tensor_copy(cp_s[t][:], cps[:])
                    else:
                        nc.scalar.copy(cp_s[t][:], cps[:])

                # ---- B2: out rows[rc] = sum_{t,kb} C_t^T[kb][:,rc]^T @ qcT_t[kb]
                # one psum tile per 512-col bank so accumulation groups are
                # fully independent (no WAR serialization via a shared tile);
                # evac + out-DMA pipeline under the remaining matmuls
                stats = [(t, kb) for t in TS for kb in range(2)]
                for rc in range(2):
                    for jc in range(4):
                        ops = psb.tile([128, 512], F32, tag=f"ob{jc}", bufs=1,
                                       name=f"ops_{jc}")
                        for qi, (t, kb) in enumerate(stats):
                            stat = cp_s[t][:, P * kb + 128 * rc:P * kb + 128 * (rc + 1)]
                            nc.tensor.matmul(ops[:], stat,
                                             qcT_s[t][:, N * kb + 512 * jc:N * kb + 512 * (jc + 1)],
                                             start=(qi == 0), stop=(qi == len(stats) - 1))
                        dst = outs[:, N * rc + 512 * jc:N * rc + 512 * (jc + 1)]
                        if jc % 2 == 0:
                            nc.vector.tensor_copy(dst, ops[:])
                        else:
                            nc.scalar.copy(dst, ops[:])
                        nc.sync.dma_start(out_d[:, N * rc + 512 * jc:N * rc + 512 * (jc + 1)], dst)

    nc.compile()
    _NC_CACHE["nc"] = nc
    return nc


def _run(X, Y, trace=False):
    _install_ntff_hook()
    import ml_dtypes
    from concourse.bass_utils import run_bass_kernel_spmd

    bf = ml_dtypes.bfloat16
    X = np.asarray(X, dtype=np.float32)
    Y = np.asarray(Y, dtype=np.float32)
    consts = _host_constants()
    Xp = np.zeros((N, N + 2), np.float32); Xp[:, 1:-1] = X
    Yp = np.zeros((N, N + 2), np.float32); Yp[:, 1:-1] = Y
    Xp = Xp.astype(bf); Yp = Yp.astype(bf)

    in_maps = []
    for c in range(NC):
        m = {"X": _to_sb(np.ascontiguousarray(Xp[:, P * c:P * c + PW]), RC),
             "Y": _to_sb(np.ascontiguousarray(Yp[:, P * c:P * c + PW]), RC)}
        m["qcall"] = consts["qcall"]
        for t in TS:
            qcf = consts[f"qcf_{t}"]
            m[f"qcT_{t}"] = consts[f"qcT_{t}"]
            m[f"qrows_{t}"] = _to_sb(qcf[P * c:P * (c + 1), :].astype(bf), 2)
            m[f"qrT_{t}"] = _to_sb(
                np.ascontiguousarray(qcf[P * c:P * (c + 1), :].T).astype(bf), 2)
            m[f"w99_{t}"] = consts[f"w99_{t}"]
        for k in ("smid", "sup", "sdn"):
            m[k] = consts[k]
        in_maps.append(m)

    nc = _build()
    r = run_bass_kernel_spmd(nc, in_maps, core_ids=list(range(NC)), trace=trace)
    blocks = []
    for c in range(NC):
        o = np.asarray(r.results[c]["out"], dtype=np.float32)  # [128, 2*2048]
        blocks.append(o.reshape(128, 2, N).transpose(1, 0, 2).reshape(2 * 128, N))
    full = np.concatenate(blocks, axis=0)
    return full[None, None], r


def kernel(X, Y):
    out, _ = _run(X, Y, trace=False)
    return out
